# revision 1
# baseline (speedup 1.0000x reference)
"""Cosine-similarity (2-slot Hungarian-matched) loss on 8 Trainium2 cores.

Math (per sample b, slots i,j in {0,1}):
    cos[i,j] = <pred[b,i]/|pred[b,i]|, gt[b,j]/|gt[b,j]|>
    best = max(cos00+cos11, cos01+cos10)
    loss = mean_b(1 - best/2)

Distribution: pure data parallel — B=32768 is split into 8 shards of 4096.
Each core streams its shard through SBUF in 16 tiles of 256 samples
([128 partitions x 4096 f32] per tensor, 2 samples per partition, 2 MiB
per DMA).  Per sample the ScalarE computes the 4 squared norms with fused
Square+accumulate, the VectorE computes the 4 cross dot products with
fused TENSOR_TENSOR_REDUCE.  A tiny epilogue normalizes
(cos = c * exp(-0.5*ln(np*ng))), picks max(id, swap) and reduces to a
[128,1] per-core partial sum of best_sum.  The host adds the 8*128
partials and finishes 1 - total/(2B).
"""

import sys

import numpy as np

sys.path.insert(0, "/opt/trn_rl_repo")

import concourse.bacc as bacc
import concourse.bass as bass
import concourse.mybir as mybir
import concourse.tile as tile
from concourse.bass_utils import run_bass_kernel_spmd

B, S, D = 32768, 2, 1024
N_CORES = 8
B_C = B // N_CORES          # samples per core
NPART = 128
TILE_S = 128                # samples per SBUF tile
NSUB = TILE_S // NPART      # samples per partition
NT = B_C // TILE_S          # tiles per core
NCOL = NT * NSUB            # stat columns per partition
F32 = mybir.dt.float32
AF = mybir.ActivationFunctionType
ALU = mybir.AluOpType


def build_nc(b_c=B_C, tile_s=TILE_S, input_bufs=6, repeat=1, dyn_repeat=0, do_act=True, do_dve=True, do_epi=True):
    nsub = tile_s // NPART
    nt = b_c // tile_s
    ncol = nt * nsub

    nc = bacc.Bacc(trn_type="TRN2")
    pred_h = nc.declare_dram_parameter("pred", [b_c, S, D], F32, isOutput=False)
    gt_h = nc.declare_dram_parameter("gt", [b_c, S, D], F32, isOutput=False)
    out_h = nc.declare_dram_parameter("out", [NPART, 1], F32, isOutput=True)

    # tile i, partition p holds samples (i*tile_s + p*nsub + j), j<nsub, each
    # a contiguous s*d run -> per-partition rows are nsub*S*D contiguous f32.
    pred_ap = pred_h[:].rearrange("(t p n) s d -> t p (n s d)", p=NPART, n=nsub)
    gt_ap = gt_h[:].rearrange("(t p n) s d -> t p (n s d)", p=NPART, n=nsub)

    with tile.TileContext(nc) as tc:
        with (
            tc.tile_pool(name="pin", bufs=input_bufs) as pin,
            tc.tile_pool(name="stats", bufs=1) as stats,
            tc.tile_pool(name="scratch", bufs=1) as scratch,
            tc.tile_pool(name="epi", bufs=1) as epi,
        ):
            # norms (ACT-written): regions [np0 | np1 | ng0 | ng1], each ncol
            # crosses (DVE-written): regions [c00 | c01 | c10 | c11], each ncol
            st_n = stats.tile([NPART, 4 * ncol], F32, tag="st_n", name="st_n")
            st_c = stats.tile([NPART, 4 * ncol], F32, tag="st_c", name="st_c")

            # Pre-load ACT table set 6 (natural_log_exp_and_others): it holds
            # square+ln+exp, so the whole kernel runs off one table load
            # instead of the 0->5->0 bounce the greedy inserter would emit.
            nc.scalar.add_instruction(
                mybir.InstLoadActFuncSet(
                    name=nc.get_next_instruction_name(),
                    act_func_set_id=6,
                    ins=[],
                    outs=[],
                )
            )
            scr_a = scratch.tile([NPART, D], F32, tag="scr_a", name="scr_a")
            scr_v = scratch.tile([NPART, D], F32, tag="scr_v", name="scr_v")

            import contextlib
            loop_cm = tc.For_i(0, dyn_repeat, 1) if dyn_repeat else contextlib.nullcontext()
            with loop_cm:
              for i in range(nt * repeat):
                  i = i % nt
                  p_t = pin.tile([NPART, nsub * S * D], F32, tag="P", name="P")
                  g_t = pin.tile([NPART, nsub * S * D], F32, tag="G", name="G")
                  nc.sync.dma_start(out=p_t[:], in_=pred_ap[i])
                  nc.sync.dma_start(out=g_t[:], in_=gt_ap[i])
                  for j in range(nsub):
                      col = i * nsub + j
                      p0 = p_t[:, (j * S + 0) * D:(j * S + 1) * D]
                      p1 = p_t[:, (j * S + 1) * D:(j * S + 2) * D]
                      g0 = g_t[:, (j * S + 0) * D:(j * S + 1) * D]
                      g1 = g_t[:, (j * S + 1) * D:(j * S + 2) * D]
                      for reg, src in (((0, p0), (1, p1), (2, g0), (3, g1)) if do_act else ()):
                          c0 = reg * ncol + col
                          nc.scalar.activation(
                              scr_a[:], src, AF.Square,
                              accum_out=st_n[:, c0:c0 + 1],
                          )
                      for reg, a, b in ((
                          (0, p0, g0), (1, p0, g1),
                          (2, p1, g0), (3, p1, g1),
                      ) if do_dve else ()):
                          c0 = reg * ncol + col
                          nc.vector.scalar_tensor_tensor(
                              out=scr_v[:], in0=a, scalar=1.0, in1=b,
                              op0=ALU.mult, op1=ALU.mult,
                              accum_out=st_c[:, c0:c0 + 1],
                          )

            # epilogue: wide fused ops over the contiguous stat regions.
            # t_all regions (i,j) follow st_c's (c00, c01, c10, c11) order.
            if not do_epi:
                nc.sync.dma_start(out=out_h[:], in_=st_n[:, 0:1])
            else:
                t_all = epi.tile([NPART, 4 * ncol], F32, tag="t_all", name="t_all")
                for idx, (i_, j_) in enumerate(((0, 0), (0, 1), (1, 0), (1, 1))):
                    nc.vector.tensor_mul(
                        t_all[:, idx * ncol:(idx + 1) * ncol],
                        st_n[:, i_ * ncol:(i_ + 1) * ncol],
                        st_n[:, (2 + j_) * ncol:(3 + j_) * ncol],
                    )
                # rsqrt via exp(-0.5*ln(t)); in-place wide ACT ops
                nc.scalar.activation(t_all[:], t_all[:], AF.Ln)
                nc.scalar.activation(t_all[:], t_all[:], AF.Exp, scale=-0.5)
                cos_all = epi.tile([NPART, 4 * ncol], F32, tag="cos_all", name="cos_all")
                nc.vector.tensor_mul(cos_all[:], st_c[:], t_all[:])
                s_id = epi.tile([NPART, ncol], F32, tag="s_id", name="s_id")
                s_sw = epi.tile([NPART, ncol], F32, tag="s_sw", name="s_sw")
                nc.vector.tensor_add(s_id[:], cos_all[:, 0:ncol], cos_all[:, 3 * ncol:4 * ncol])
                nc.vector.tensor_add(s_sw[:], cos_all[:, ncol:2 * ncol], cos_all[:, 2 * ncol:3 * ncol])
                best = epi.tile([NPART, ncol], F32, tag="best", name="best")
                partial = epi.tile([NPART, 1], F32, tag="partial", name="partial")
                nc.vector.tensor_max(best[:], s_id[:], s_sw[:])
                nc.vector.reduce_sum(partial[:], best[:], axis=mybir.AxisListType.X)
                nc.sync.dma_start(out=out_h[:], in_=partial[:])
    nc.finalize()
    return nc


_CACHE = {}


def _get_nc():
    if "nc" not in _CACHE:
        _CACHE["nc"] = build_nc()
    return _CACHE["nc"]


def run_spmd(pred, gt, **kwargs):
    """Run the SPMD kernel; returns (BassKernelResults, per-core partials)."""
    pred = np.ascontiguousarray(np.asarray(pred), dtype=np.float32)
    gt = np.ascontiguousarray(np.asarray(gt), dtype=np.float32)
    assert pred.shape == (B, S, D) and gt.shape == (B, S, D)
    nc = _get_nc()
    in_maps = [
        {"pred": pred[c * B_C:(c + 1) * B_C], "gt": gt[c * B_C:(c + 1) * B_C]}
        for c in range(N_CORES)
    ]
    res = run_bass_kernel_spmd(nc, in_maps, list(range(N_CORES)), **kwargs)
    return res


def kernel(pred, gt):
    res = run_spmd(pred, gt)
    total = sum(
        float(np.sum(r["out"], dtype=np.float64)) for r in res.results
    )
    loss = 1.0 - total / (2.0 * B)
    return np.array(loss, dtype=np.float32)

